# revision 18
# baseline (speedup 1.0000x reference)
"""MatAnyone memory-readout kernel for 8 Trainium2 NeuronCores (fp8 DoubleRow).

Math (per batch b, query pixel n, memory slot t):
  sim[t,n] = ms[t]*(-a_sq + 2ab - b_sq)[t,n]/sqrt(CK)
  aff      = softmax_t(sim);  R[c,n] = sum_t mv[c,t]*aff[t,n]
  out[c,n] = R[c,n]*p[n] + lv[c,n]*(1-p[n])

Sharding: 8 cores = 2 batches x 4 query-pixel shards (n = 576 per core).

Per-core plan (fp8 e4m3 DoubleRow matmuls; DR = K-paired contraction,
out = sum_i W[:,i].T @ X[:,i], both operands fp8):
  sim: Ki=65 padded to 128. lhsT pairs = [ms*mk^2/2 | 2*ms*mk] per channel
       plus a 65th row (2ms | ms/4) that folds in -b_sq*ms/8 (fp8 residual
       correction in the second slot). rhs pairs = [-qe/2 | qe*qk/4] plus
       (-b_sq/8 | 8*residual). One DR matmul per (t-tile, n-half) writes
       psum = 2*sim; halves live at 512-aligned offsets (psum bank-crossing
       matmul writes are broken on trn2 - everything is bank-aligned).
  exp: tile j=0 of each pair on ACT (E = exp(0.5*psum) -> fp8); tile j=1 on
       DVE via a one-instruction Schraudolph straight to e4m3 bits:
       u8 = sat_rne(A*psum + B), A = 4/ln2, B ~ 55.65, bitcast u8 -> fp8.
       (f32->u8 DVE conversion saturates negatives to 0 = fp8 +0.0; sim<=0
       keeps the high side far from the 0x7F NaN encoding.)
  R:   t-tiles paired (K=256 over t): lhsT = mv pair-chunk [128,2,128],
       rhs = E-pair [128,2,288]. 4 DR matmuls per pair, accumulated over
       all 72 pairs in psum.
  Z:   per pair, 3 DR ones-weight matmuls (M=1, N=192) into the gap columns
       (s*512+288) of the R psum banks, accumulated over pairs at lag 1 so
       the normalization chain can start before the last readouts finish.
  PSUM (8 banks): RZ tile [128,2048] = R quarters @ bank starts (cols
       (2k+hh)*512) + Z segs @ s*512+288 (3 banks); sim pool [128,1024] x 2.
  Warmup matmuls on memset-initialized tiles (no DMA dependency) trip the
  HAM clock-gate to 8/8 during the initial DMA wait.
  out = R*(2p/2Z) + lv*(1-p) in bf16; lv*(1-p) precomputed on host.
"""

import sys

for _p in ("/opt/trn_rl_repo", "/root/.axon_site/_ro/trn_rl_repo"):
    if _p not in sys.path:
        sys.path.insert(0, _p)

from contextlib import ExitStack

import numpy as np
import ml_dtypes

import concourse.bass as bass
from concourse import mybir
from concourse.bacc import Bacc
from concourse.tile import TileContext
from concourse.bass_utils import run_bass_kernel_spmd

F32 = mybir.dt.float32
U8 = mybir.dt.uint8
BF16 = mybir.dt.bfloat16
FP8 = mybir.dt.float8e4
EXP = mybir.ActivationFunctionType.Exp
CPY = mybir.ActivationFunctionType.Copy
DR = mybir.MatmulPerfMode.DoubleRow
DRS = mybir.MatmulPerfMode.DoubleRowSwInterleave
E4M3 = ml_dtypes.float8_e4m3

B, CK, CV, T, H, W = 2, 64, 256, 8, 48, 48
HW = H * W            # 2304
THW = T * HW          # 18432
NCORE = HW // 4       # 576 query pixels per core
NH = NCORE // 2       # 288 per n-half
TT = THW // 128       # 144 t-tiles
NPAIR = TT // 2       # 72 t-tile pairs
SKEW = 3              # pairs of lag between exp and readout
CHP = 8               # pairs per streamed mkw chunk
ZSEG = 192            # Z segment width (3 segments of 192 = 576)
NWARM = 32            # HAM warmup matmuls

# Schraudolph-to-e4m3: bits = A*psum + B so that value ~= exp(0.5*psum)
SCH8_A = 4.0 / float(np.log(2.0))   # 5.7708
SCH8_B = 55.65                      # 56 (e4m3 bias*8) + centering tweak

_CACHE = {}


def _fp8(x):
    return np.clip(x, -240.0, 240.0).astype(E4M3)


def build_program():
    nc = Bacc(name="matanyone_fp8dr")

    qw_h = nc.declare_dram_parameter("qw", [128, 2 * NCORE], FP8, isOutput=False)
    mkw_h = nc.declare_dram_parameter("mkw", [128, TT * 256], FP8,
                                      isOutput=False)
    mvw_h = nc.declare_dram_parameter("mvw", [128, NPAIR * 512], FP8,
                                      isOutput=False)
    lvw2_h = nc.declare_dram_parameter("lvw2", [CV, NCORE], BF16,
                                       isOutput=False)
    p_h = nc.declare_dram_parameter("p", [1, NCORE], F32, isOutput=False)
    out_h = nc.declare_dram_parameter("out", [CV, NCORE], BF16, isOutput=True)

    with TileContext(nc) as tc, ExitStack() as ctx:
        persist = ctx.enter_context(tc.tile_pool(name="persist", bufs=1))
        ps_rz0 = ctx.enter_context(tc.tile_pool(name="psrz", bufs=1,
                                                space="PSUM"))
        mvpool = ctx.enter_context(tc.tile_pool(name="mv", bufs=1))
        m2pool = ctx.enter_context(tc.tile_pool(name="m2", bufs=3))
        epool = ctx.enter_context(tc.tile_pool(name="E", bufs=SKEW + 2))
        ps_sim = ctx.enter_context(tc.tile_pool(name="pssim", bufs=4,
                                                space="PSUM"))
        fin = ctx.enter_context(tc.tile_pool(name="fin", bufs=1))

        # RZ: R quarters (k,hh) @ (2k+hh)*512 + Z segs @ s*512+288
        rz = ps_rz0.tile([128, 2048], F32, tag="rz")

        # ---- first-pair DMA issues ride ahead of everything ----------------
        qw = persist.tile([128, 2 * NCORE], FP8, tag="qw")
        mkcs = {}
        mkc0 = m2pool.tile([128, CHP * 512], FP8, tag="mkc")
        mkcs[0] = mkc0
        nc.sync.dma_start(out=mkc0[:, 0:512], in_=mkw_h[:, 0:512])
        nc.gpsimd.dma_start(out=qw[:, 0:NCORE], in_=qw_h[:, 0:NCORE])
        nc.scalar.dma_start(out=qw[:, NCORE:2 * NCORE],
                            in_=qw_h[:, NCORE:2 * NCORE])
        nc.sync.dma_start(out=mkc0[:, 512:1536], in_=mkw_h[:, 512:1536])
        nc.sync.dma_start(out=mkc0[:, 1536:CHP * 512],
                          in_=mkw_h[:, 1536:CHP * 512])

        # ---- constants via memset (no DMA dependency) ----------------------
        onesz = persist.tile([128, 32], FP8, tag="onesz")
        nc.vector.memset(onesz[:], 1.0)
        wrm = persist.tile([128, 256], FP8, tag="wrm")
        nc.vector.memset(wrm[:], 1.0)
        onesb1 = persist.tile([1, 128], BF16, tag="onesb1")
        nc.vector.memset(onesb1[:], 1.0)
        actw = persist.tile([1, 16], F32, tag="actw")
        nc.vector.memset(actw[:], 0.0)
        actwo = persist.tile([1, 16], F32, tag="actwo")
        # preload the exp table set while DMAs stream in
        nc.scalar.activation(actwo[:], actw[:], EXP, scale=0.5)

        onesz3 = onesz.rearrange("p (i m) -> p i m", i=2)
        wrm3 = wrm.rearrange("p (i n) -> p i n", i=2)   # [128, 2, 128]

        # PE warmup on memset tiles: keeps PE busy from ~6.3us so the HAM
        # clock-gate flips to 8/8 before the first real matmuls.
        for _w in range(NWARM):
            nc.tensor.matmul(rz[0:1, 288:416], onesz3[:, :, 0:1],
                             wrm3[:], start=True, stop=True, perf_mode=DR)

        # ---- input DMAs ----------------------------------------------------
        qw3 = qw.rearrange("p (i n) -> p i n", i=2)          # [128, 2, 576]
        mvw = mvpool.tile([128, NPAIR * 512], FP8, tag="mvw")
        nc.scalar.dma_start(out=mvw[:, 0:CHP * 512],
                            in_=mvw_h[:, 0:CHP * 512])
        NCHUNK = NPAIR // CHP

        def issue_chunk(g):
            if g in mkcs or g >= NCHUNK:
                return
            t = m2pool.tile([128, CHP * 512], FP8, tag="mkc")
            mkcs[g] = t
            nc.sync.dma_start(
                out=t[:],
                in_=mkw_h[:, g * CHP * 512:(g + 1) * CHP * 512])
            nc.gpsimd.dma_start(
                out=mvw[:, g * CHP * 512:(g + 1) * CHP * 512],
                in_=mvw_h[:, g * CHP * 512:(g + 1) * CHP * 512])

        # finalize inputs, prefetched behind the first-pair DMAs
        p_sb = persist.tile([1, NCORE], F32, tag="p")
        nc.gpsimd.dma_start(out=p_sb[:], in_=p_h[:])
        lvw2 = []
        for k in (0, 1):
            t = persist.tile([128, NCORE], BF16, tag=f"lvw2{k}")
            nc.gpsimd.dma_start(out=t[:], in_=lvw2_h[k * 128:(k + 1) * 128, :])
            lvw2.append(t)

        e_tiles = {}

        def pair_front(a):
            if a % CHP == CHP // 2:
                issue_chunk(a // CHP + 1)
            mkc = mkcs[a // CHP]
            e = epool.tile([128, 2 * NCORE], FP8, tag="E")
            eu = e.bitcast(U8)
            e_tiles[a] = e

            # sim + exp at (tile, half) granularity: each matmul writes a
            # private 1-bank psum tile so the four psum bufs rotate with a
            # full pair of slack between PE write and exp read-back.
            for j in (0, 1):
                wsl = mkc[:, (a % CHP) * 512 + j * 256:(a % CHP) * 512
                          + (j + 1) * 256]
                w3 = wsl.rearrange("p (m i) -> p i m", i=2)
                for hh in (0, 1):
                    sim = ps_sim.tile([128, 512], F32, tag="sim")
                    nc.tensor.matmul(sim[:, 0:NH], w3,
                                     qw3[:, :, hh * NH:(hh + 1) * NH],
                                     start=True, stop=True, perf_mode=DRS)
                    off = j * NCORE + hh * NH
                    if hh == 1:
                        # one-instruction Schraudolph exp on DVE, straight
                        # to e4m3 bits (f32->u8 is RNE + saturating at 0)
                        nc.vector.tensor_scalar(eu[:, off:off + NH],
                                                sim[:, 0:NH],
                                                SCH8_A, SCH8_B,
                                                mybir.AluOpType.mult,
                                                mybir.AluOpType.add)
                    else:
                        nc.scalar.activation(e[:, off:off + NH], sim[:, 0:NH],
                                             EXP, scale=0.5)

        def pair_back_k(a, k):
            e = e_tiles[a] if k == 0 else e_tiles.pop(a)
            e3 = e.rearrange("p (i n) -> p i n", i=2)
            st, sp = (a == 0), (a == NPAIR - 1)
            wsl = mvw[:, a * 512 + k * 256:a * 512 + (k + 1) * 256]
            w3 = wsl.rearrange("p (m i) -> p i m", i=2)
            for hh in (0, 1):
                q = (2 * k + hh) * 512
                nc.tensor.matmul(
                    rz[:, q:q + NH],
                    w3, e3[:, :, hh * NH:(hh + 1) * NH],
                    start=st, stop=sp, perf_mode=DRS)

        def pair_back_z(a):
            e3 = e_tiles[a].rearrange("p (i n) -> p i n", i=2)
            for s in range(3):
                nc.tensor.matmul(
                    rz[0:1, s * 512 + 288:s * 512 + 288 + ZSEG],
                    onesz3[:, :, 0:1], e3[:, :, s * ZSEG:(s + 1) * ZSEG],
                    start=(a == 0), stop=(a == NPAIR - 1), perf_mode=DR)

        # per iteration: guaranteed-ready back-work first (Z at lag 2,
        # readout at lag SKEW) so exp latency never stalls PE's in-order
        # queue, then the new pair's sim matmuls.
        for a in range(NPAIR + SKEW):
            if 2 <= a < NPAIR + 2:
                pair_back_z(a - 2)
            if a >= SKEW:
                pair_back_k(a - SKEW, 0)
                pair_back_k(a - SKEW, 1)
            if a < NPAIR:
                pair_front(a)

        # ---- finalize ------------------------------------------------------
        # 1/(2Z) on ACT straight from psum (p has 2x folded in on host)
        rzv = fin.tile([1, NCORE], F32, tag="rzv")
        rz4 = rz.rearrange("p (s c) -> p s c", c=512)[0:1, 0:3, 288:288 + ZSEG]
        eng = nc.scalar
        eng.add_instruction(mybir.InstActivation(
            name=nc.get_next_instruction_name(),
            func=mybir.ActivationFunctionType.Reciprocal,
            ins=[eng.lower_ap(rz4),
                 mybir.ImmediateValue(dtype=mybir.dt.float32, value=0.0),
                 mybir.ImmediateValue(dtype=mybir.dt.float32, value=2.0),
                 mybir.ImmediateValue(dtype=mybir.dt.float32, value=0.0)],
            outs=[eng.lower_ap(rzv.rearrange("p (s n) -> p s n", s=3))]))
        # per half: w1 = 2p/2Z on DVE, broadcast via ones matmul, copy on ACT
        w1 = fin.tile([1, NCORE], BF16, tag="w1")
        w1s = fin.tile([128, NCORE], F32, tag="w1s")
        for hh in (0, 1):
            sl = slice(hh * NH, (hh + 1) * NH)
            nc.vector.tensor_mul(w1[:, sl], rzv[:, sl], p_sb[:, sl])
            wt = ps_sim.tile([128, 512], F32, tag="sim")
            nc.tensor.matmul(wt[:, 0:NH], onesb1[:], w1[:, sl],
                             start=True, stop=True)
            nc.scalar.activation(w1s[:, sl], wt[:, 0:NH], CPY)

        # blend + store at (k, hh) granularity so DVE muls pipeline with
        # the ACT copies and the out DMAs start as soon as possible
        dmae = [nc.sync, nc.scalar, nc.gpsimd, nc.sync]
        for k in (0, 1):
            o = fin.tile([128, NCORE], BF16, tag=f"o{k}")
            for hh in (0, 1):
                q = (2 * k + hh) * 512
                osl = o[:, hh * NH:(hh + 1) * NH]
                nc.vector.tensor_mul(osl, rz[:, q:q + NH],
                                     w1s[:, hh * NH:(hh + 1) * NH])
                nc.vector.tensor_add(osl, osl,
                                     lvw2[k][:, hh * NH:(hh + 1) * NH])
                dmae[2 * k + hh].dma_start(
                    out=out_h[k * 128:(k + 1) * 128,
                              hh * NH:(hh + 1) * NH],
                    in_=osl)

    nc.finalize()
    return nc


def _get_program():
    if "nc" not in _CACHE:
        _CACHE["nc"] = build_program()
    return _CACHE["nc"]


def _make_in_maps(query_key, query_selection, memory_key, memory_shrinkage,
                  msk_value, uncert_prob):
    qk = np.asarray(query_key, np.float32).reshape(B, CK, HW)
    qe = np.asarray(query_selection, np.float32).reshape(B, CK, HW)
    mk = np.asarray(memory_key, np.float32).reshape(B, CK, THW)
    ms = np.asarray(memory_shrinkage, np.float32).reshape(B, THW)
    mv = np.asarray(msk_value, np.float32).reshape(B, CV, THW)
    lv = np.asarray(msk_value, np.float32).reshape(B, CV, T, HW)[:, :, T - 1, :]
    p = np.asarray(uncert_prob, np.float32).reshape(B, HW)

    # per-batch sim weights: [65, TT, 2, 128]
    mkw_b = []
    mvw_b = []
    for b in range(B):
        mk3 = mk[b].reshape(CK, TT, 128)               # [c, tau, m]
        ms3 = ms[b].reshape(TT, 128)                   # [tau, m]
        mkw = np.zeros((128, TT, 2, 128), np.float32)
        mkw[:CK, :, 0, :] = ms3[None] * mk3 * mk3 * 0.5
        mkw[:CK, :, 1, :] = 2.0 * ms3[None] * mk3
        mkw[64, :, 0, :] = 2.0 * ms3
        mkw[64, :, 1, :] = 0.25 * ms3
        mkw_swi = mkw[:, :, :, ::-1].transpose(0, 1, 3, 2)  # [p,t,m_rev,i]
        mkw_b.append(_fp8(mkw_swi).reshape(128, TT * 256))
        # mv DR weights: [p, a, k, i, m] = mv[k*128+m, (2a+i)*128+p]
        tmp = mv[b].reshape(2, 128, NPAIR, 2, 128)     # [k, m, a, i, p]
        mvw = tmp[:, ::-1].transpose(4, 2, 0, 1, 3)     # [p, a, k, m_rev, i]
        mvw_b.append(_fp8(mvw.reshape(128, NPAIR * 512)))

    in_maps = []
    for core in range(8):
        b, s = divmod(core, 4)
        sl = slice(s * NCORE, (s + 1) * NCORE)
        qks, qes = qk[b, :, sl], qe[b, :, sl]
        bsq = np.einsum("cn,cn->n", qes, qks * qks)    # [576]
        qwf = np.zeros((128, 2, NCORE), np.float32)
        qwf[:CK, 0] = -0.5 * qes
        qwf[:CK, 1] = 0.25 * qes * qks
        b0 = _fp8(-bsq / 8.0)
        qwf[64, 0] = b0.astype(np.float32)
        qwf[64, 1] = 8.0 * (-bsq / 8.0 - b0.astype(np.float32))
        ps = p[b, sl]
        in_maps.append({
            "qw": _fp8(qwf).reshape(128, 2 * NCORE),
            "mkw": mkw_b[b],
            "mvw": mvw_b[b],
            "lvw2": np.ascontiguousarray(
                lv[b, :, sl] * (1.0 - ps)[None, :]).astype(ml_dtypes.bfloat16),
            "p": np.ascontiguousarray(2.0 * ps).reshape(1, NCORE),
        })
    return in_maps


def kernel(**inputs):
    nc = _get_program()
    in_maps = _make_in_maps(**inputs)
    res = run_bass_kernel_spmd(nc, in_maps, list(range(8)))
    out = np.empty((B, 1, CV, HW), np.float32)
    for core in range(8):
        b, s = divmod(core, 4)
        out[b, 0, :, s * NCORE:(s + 1) * NCORE] = np.asarray(
            res.results[core]["out"], dtype=np.float32)
    return out.reshape(B, 1, CV, H, W)


if __name__ == "__main__":
    rng = np.random.default_rng(0)
    dummy = {
        "query_key": rng.standard_normal((B, CK, H, W)).astype(np.float32),
        "query_selection": rng.random((B, CK, H, W)).astype(np.float32),
        "memory_key": rng.standard_normal((B, CK, T, H, W)).astype(np.float32),
        "memory_shrinkage": rng.random((B, 1, T, H, W)).astype(np.float32),
        "msk_value": rng.standard_normal((B, 1, CV, T, H, W)).astype(np.float32),
        "uncert_prob": rng.random((B, 1, H, W)).astype(np.float32),
    }
    out = kernel(**dummy)
    print("out", out.shape, out.dtype, float(np.abs(out).mean()))
